# revision 54
# baseline (speedup 1.0000x reference)
"""Trainium2 Bass kernel for nn_Joint (dense transformer block), 8 NeuronCores.

Sharding: 8 cores = 4 batches x 2 sequence halves. Each core computes the
full MLP->h and K/V projections for its batch (duplicated inside the pair,
no collectives), but only its own 1024-token half of queries / attention
rows / FFN / output. Token "roll" trick: each core's x is rotated so its own
half is always tokens [0:1024]; attention over all 2048 keys is
permutation-invariant, so the same SPMD program works for both halves.

v12: fp8 DoubleRow tensor math everywhere the numerics allow.
Q/K/V projections, scores, attn@V and both FFN layers run as float8e4
DoubleRow matmuls (lhsT [K,2,M], rhs [K,2,N], 2 contraction k-tiles per
pass = 2x bf16 throughput on HW). The HID=568 contraction is zero-padded
to 768 = 3 k-tile pairs; D=1024 contractions are 4 pairs. Weights are
host-prescaled x32 (FFN x32/x64) and activations x4 to sit in e4m3's
normal range; scales unwind in the PSUM-drain activations and the
attention rowsum trick (ones=4 folds the V scale into the softmax
denominator). wv is quantized with host-side DITHERING (greedy rounding
flips per column zeroing the error column-sums): plain rounding leaves a
coherent bias that attention's averaging cannot remove. MLP and x_mod
stay bf16 (their weight/activation quantization error lands coherently
in the LayerNorm'd residual). The LN1 affine is folded into wf1/bf1 on
the host; when both LN affines are exactly identity (checked at runtime)
a specialized program fuses LN2-normalize+relu into one activation.
PSUM drains are split across scalar/vector so fp8-sized matmul tiles
stay PE-bound instead of drain-bound; a dozen dummy matmuls pre-ramp the
PE clock during the initial DMA wait, and every big DMA is split across
the three DMA-capable queues (each ring tops out near ~100GB/s).
Transposed-scores attention as in v2: P^T = exp(S^T*scale) lands in SBUF
as attn's lhsT; rowsums via an N=2 matmul against a constant fp8 tile.

Layouts on chip (per core):
  xT    [768, 2048]   bf16 feature-major (host pre-transposed)
  hT    [568(+1), 2048] bf16; row 56 of chunk 4 = ones (bias row for wm)
  h8    3x[128, 2, 2048] fp8 pair-tiles (chunk 2j+i in slot i; tail zero)
  kT    4x[128, 2, 2048] fp8 pair-tiles (d-chunk 2j+i), prescaled 4x
  qT    4x[128, 2, 1024] fp8 pair-tiles, prescaled 4x
  V     8x[128, 2, 1024] fp8 pair-tiles (token block 2j+i), prescaled 4x
  xmod  [1024, 1024] bf16 token-major, bm+bv folded in via the ones row
  PT    [2048, 512]  fp8  exp(S^T/512) per 512-query block
  x1    [1024, 1024] bf16 token-major holds t=(x1pre-mu)*rstd (pre-affine)
  x1T   [1024, 1024] fp8  4*t via PE transpose + x4 drain, feeds FFN
  f1T   [1024, 1024] fp8  4*relu(t@wf1g+bf1g) per 512-token chunk
All fp8 matmuls accumulate in fp32 PSUM; softmax/LN math fp32.
"""

import sys

if "/opt/trn_rl_repo" not in sys.path:
    sys.path.insert(0, "/opt/trn_rl_repo")

import numpy as np
import ml_dtypes

import concourse.bass as bass
import concourse.mybir as mybir
import concourse.tile as tile
from concourse import bacc
from concourse.masks import make_identity

BF16 = mybir.dt.bfloat16
FP8 = mybir.dt.float8e4
F32 = mybir.dt.float32
AF = mybir.ActivationFunctionType
ALU = mybir.AluOpType
AX = mybir.AxisListType
DR = mybir.MatmulPerfMode.DoubleRow

B, S, IN_C, HID, D = 4, 2048, 768, 568, 1024
Q = S // 2  # own-half query tokens per core
KB = S // 128  # 16 key blocks
EPS = 1e-5
SCALE = 1.0 / np.sqrt(np.float32(D))  # 1/32
NCORES = 8

# K-chunking of the HID=568 contraction (bf16 wm path): 4x128 + 56(+1 ones)
HID_CH = [128, 128, 128, 128, 56]
HID_CH_AUG = [128, 128, 128, 128, 57]

# fp8 scale plan: weights x32, q/k/v activations x4
WS = 32.0  # wq/wk/wv host prescale
QS = 4.0  # q/k/v activation prescale
DRAIN_SCALE = QS / WS  # 1/8, applied when draining fp8 projection PSUM
EXP_SCALE = float(SCALE) / (QS * QS)  # scores PSUM is (4q).(4k) = 16 qk


def build_program(identity_affine=False):
    """identity_affine=True specializes for g1=g2=1, be1=be2=0 (checked on
    the host): the LN affines collapse to a single bf2 add on the residual
    and the LN2 normalize fuses into the final Relu activation."""
    nc = bacc.Bacc("TRN2")

    # ---- DRAM I/O ----
    xT = nc.dram_tensor("xT", [IN_C, S], BF16, kind="ExternalInput")
    w_mlp = nc.dram_tensor("w_mlp", [IN_C, HID], BF16, kind="ExternalInput")
    # fp8 projection weights, host-prescaled x32, zero-padded 568->768 rows,
    # pre-arranged as [pair j, partition p, slot i, d] with row = 256j+128i+p
    wq8 = nc.dram_tensor("wq8", [3, 128, 2, D], FP8, kind="ExternalInput")
    wk8 = nc.dram_tensor("wk8", [3, 128, 2, D], FP8, kind="ExternalInput")
    # wv in fp8 with host-side dithered quantization: plain rounding leaves a
    # coherent per-column bias that attention's averaging cannot remove;
    # dithering zeroes the column sums and cuts that bias ~10x
    wv8 = nc.dram_tensor("wv8", [3, 128, 2, D], FP8, kind="ExternalInput")
    # wm is host-augmented to HID+1 rows carrying bm+bv -- the hT ones-row
    # turns that into a free bias add
    wm = nc.dram_tensor("wm", [HID + 1, D], BF16, kind="ExternalInput")
    # fp8 FFN weights (g1 folded into wf1), [pair j, p, slot i, d]
    wf18 = nc.dram_tensor("wf18", [4, 128, 2, D], FP8, kind="ExternalInput")
    wf28 = nc.dram_tensor("wf28", [4, 128, 2, D], FP8, kind="ExternalInput")
    b_mlp = nc.dram_tensor("b_mlp", [HID], F32, kind="ExternalInput")
    bq_s = nc.dram_tensor("bq_s", [D], F32, kind="ExternalInput")  # bq*4
    bk_s = nc.dram_tensor("bk_s", [D], F32, kind="ExternalInput")  # bk*4
    bf1g = nc.dram_tensor("bf1g", [D], F32, kind="ExternalInput")  # 4*(bf1+be1@wf1)
    bf2 = nc.dram_tensor("bf2", [D], BF16, kind="ExternalInput")
    g1 = nc.dram_tensor("g1", [D], BF16, kind="ExternalInput")
    be1 = nc.dram_tensor("be1", [D], BF16, kind="ExternalInput")
    g2 = nc.dram_tensor("g2", [D], BF16, kind="ExternalInput")
    be2 = nc.dram_tensor("be2", [D], BF16, kind="ExternalInput")
    y = nc.dram_tensor("y", [Q, D], F32, kind="ExternalOutput")

    def bcast_ap(handle, n):
        a = handle[:]
        return bass.AP(tensor=a.tensor, offset=a.offset, ap=[[0, 128]] + list(a.ap))

    with tile.TileContext(nc) as tc:
        with (
            tc.tile_pool(name="singles", bufs=1) as singles,
            tc.tile_pool(name="x1_pool", bufs=1) as x1_pool,
        ):
            # ---------- pools (stack order: kqvm, hT below stream/xw) ----------
            x1T_pool = tc.alloc_tile_pool(name="x1T", bufs=1, side="right")
            # x1T holds 4*t (LN1-normalized, pre-affine) in fp8 for the FFN
            x1T = x1T_pool.tile([128, 8, Q], FP8, name="x1T")

            kqvm = tc.alloc_tile_pool(name="kqvm", bufs=1, side="left")
            kTp = [kqvm.tile([128, 2, S], FP8, tag=f"kT_{j}", name=f"kT_{j}") for j in range(4)]
            qTp = [kqvm.tile([128, 2, Q], FP8, tag=f"qT_{j}", name=f"qT_{j}") for j in range(4)]
            vp = [kqvm.tile([128, 2, D], FP8, tag=f"v_{j}", name=f"v_{j}") for j in range(8)]
            xm_sb = [kqvm.tile([128, D], BF16, tag=f"xm_{i}", name=f"xm_{i}") for i in range(8)]

            hT_pool = tc.alloc_tile_pool(name="hT", bufs=1, side="left")
            hT_sb = [hT_pool.tile([128, S], BF16, tag=f"hT_{i}", name=f"hTs_{i}") for i in range(5)]
            h8p = [hT_pool.tile([128, 2, S], FP8, tag=f"h8_{j}", name=f"h8_{j}") for j in range(3)]

            # ---------- phase 0 prologue: big DMAs issue first ----------
            stream = tc.alloc_tile_pool(name="stream", bufs=1, side="left")
            xw_pool = tc.alloc_tile_pool(name="xw", bufs=1, side="left")
            wmlp_sb = xw_pool.tile([128, 6, HID], BF16, name="wmlp_sb")

            def load_xs_wave(xs, n):
                # slice-per-chunk over three queues: per-queue DMA rings top
                # out near ~100GB/s, so spread every big transfer
                for i in range(6):
                    eng = (nc.gpsimd, nc.sync, nc.scalar)[i % 3]
                    eng.dma_start(
                        out=xs[:, i, :],
                        in_=xT[i * 128 : (i + 1) * 128, n * 512 : (n + 1) * 512],
                    )

            xs_tiles = [
                stream.tile([128, 6, 512], BF16, tag="xs", name="xs", bufs=4)
                for _ in range(4)
            ]
            # wave 0 first, then wmlp in six chunks round-robined over the
            # rings, then waves 1-3: the first matmul needs wave0 + all of
            # wmlp, and each ring moves ~550KB of that critical set
            load_xs_wave(xs_tiles[0], 0)
            for k in range(6):
                eng = (nc.gpsimd, nc.sync, nc.scalar)[k % 3]
                eng.dma_start(
                    out=wmlp_sb[:, k, :],
                    in_=w_mlp[128 * k : 128 * k + 128, :],
                )
            for n in range(1, 4):
                load_xs_wave(xs_tiles[n], n)

            # constants / biases (small; gpsimd queue, after its xs slices)
            # warm_t memset leads the vector queue so the PE warm-up isn't
            # stuck behind the big h8 tail memset
            warm_t = singles.tile([128, 128], BF16, name="warm_t")
            nc.vector.memset(warm_t, 0.0)
            ident = singles.tile([128, 128], BF16)
            make_identity(nc, ident)
            eps_t = singles.tile([128, 1], F32)
            nc.vector.memset(eps_t, EPS)
            ones8 = singles.tile([128, 2, 2], FP8)
            nc.vector.memset(ones8, QS)  # folds the V prescale into rowsum

            bmlp_sb = singles.tile([128, 5], F32)
            for m in range(5):
                m0 = m * 128
                msz = HID_CH[m]
                nc.gpsimd.dma_start(
                    out=bmlp_sb[:msz, m : m + 1],
                    in_=b_mlp[m0 : m0 + msz].rearrange("(a b) -> a b", b=1),
                )
            bq_sb = singles.tile([128, 8], F32)
            nc.gpsimd.dma_start(out=bq_sb, in_=bq_s.rearrange("(c p) -> p c", p=128))
            bk_sb = singles.tile([128, 8], F32)
            nc.gpsimd.dma_start(out=bk_sb, in_=bk_s.rearrange("(c p) -> p c", p=128))
            bf1_sb = singles.tile([128, 8], F32)
            nc.gpsimd.dma_start(out=bf1_sb, in_=bf1g.rearrange("(c p) -> p c", p=128))

            # small LN scratch
            mu4 = singles.tile([128, 4], F32)
            sq4 = singles.tile([128, 4], F32)
            ve4 = singles.tile([128, 4], F32)
            msqe4 = singles.tile([128, 4], F32)
            std4 = singles.tile([128, 4], F32)
            rstd4 = singles.tile([128, 4], F32)
            rercp4 = singles.tile([128, 4], F32)
            sa0 = singles.tile([128, 1], F32)
            sa1 = singles.tile([128, 1], F32)
            musum = singles.tile([128, 1], F32)
            sqa = singles.tile([128, 1], F32)
            mu2 = singles.tile([128, 1], F32)
            msqe2 = singles.tile([128, 1], F32)
            ve2 = singles.tile([128, 1], F32)
            std2 = singles.tile([128, 1], F32)
            rstd2 = singles.tile([128, 1], F32)

            x1_sb = [x1_pool.tile([128, D], BF16, tag=f"x1_{i}", name=f"x1_{i}") for i in range(8)]

            pp_sc = tc.alloc_tile_pool(name="psum_sc", bufs=1, space="PSUM")
            pp_mm = tc.alloc_tile_pool(name="psum_mm", bufs=1, space="PSUM")

            # ---------- phase 0: hT = relu(w_mlp.T @ xT + b_mlp) ----------
            # ones row for the bias-via-matmul trick (xm): memset a 32-aligned
            # partition band; mlp writes rows 0..55, leaving row 56 = 1.0
            # (vector: the gpsimd queue must keep streaming xT waves)
            nc.vector.memset(hT_sb[4][32:64, :], 1.0)
            # fp8 h pair-tile tail (chunk 4 rows 56.. and slot 1) must be zero
            nc.vector.memset(h8p[2], 0.0)

            # PE p-state warm-up: dummy matmuls while the first xT wave is in
            # flight, so the MLP starts at the ramped clock instead of paying
            # the slow-start tax
            for w in range(12):
                wps = pp_mm.tile([128, 512], F32, tag="mm", bufs=6)
                nc.tensor.matmul(wps[:, 0:128], warm_t, warm_t, start=True, stop=True)

            for n in range(4):
                ns = bass.ts(n, 512)
                xs = xs_tiles[n]
                if n >= 2:
                    load_xs_wave(xs, n)
                for m in range(5):
                    m0, msz = m * 128, HID_CH[m]
                    ps = pp_mm.tile([128, 512], F32, tag="mm", bufs=6)
                    for kk in range(6):
                        nc.tensor.matmul(
                            ps[:msz],
                            wmlp_sb[:, kk, m0 : m0 + msz],
                            xs[:, kk, :],
                            start=(kk == 0),
                            stop=(kk == 5),
                        )
                    nc.scalar.activation(
                        out=hT_sb[m][:msz, ns],
                        in_=ps[:msz],
                        func=AF.Relu,
                        bias=bmlp_sb[:msz, m : m + 1],
                    )
                    # fp8 copy of h for the q/k/v projections
                    nc.vector.tensor_copy(
                        h8p[m // 2][: min(msz, 128), m % 2, ns], hT_sb[m][:msz, ns]
                    )
            xw_pool.release()

            # fp8 projection weights stream behind the xT waves on sync
            def load_w8(wdram, st):
                tiles = []
                for j in range(3):
                    t = stream.tile([128, 2, D], FP8, tag=f"w8{st}_{j}", name=f"w8{st}_{j}")
                    nc.sync.dma_start(out=t, in_=wdram[j])
                    tiles.append(t)
                return tiles

            wk8_sb = load_w8(wk8, 0)
            wq8_sb = load_w8(wq8, 1)

            # ---------- phase 1: projections (fp8 DoubleRow; drains split) ----
            # n outer: iteration (n=0) only needs wave-0 h8 casts, so the PE
            # doesn't stall on the tail wave's cast at the phase boundary
            for n in range(4):
                ns = bass.ts(n, 512)
                for m in range(8):
                    ms = bass.ts(m, 128)
                    ps = pp_mm.tile([128, 512], F32, tag="mm", bufs=6)
                    for j in range(3):
                        nc.tensor.matmul(
                            ps,
                            wk8_sb[j][:, :, ms],
                            h8p[j][:, :, ns],
                            start=(j == 0),
                            stop=(j == 2),
                            perf_mode=DR,
                        )
                    dst = kTp[m // 2][:, m % 2, ns]
                    if (m * 4 + n) % 2 == 0:
                        nc.scalar.activation(
                            out=dst, in_=ps, func=AF.Identity,
                            scale=DRAIN_SCALE, bias=bk_sb[:, m : m + 1],
                        )
                    else:
                        nc.vector.tensor_scalar(
                            out=dst, in0=ps, scalar1=DRAIN_SCALE,
                            scalar2=bk_sb[:, m : m + 1], op0=ALU.mult, op1=ALU.add,
                        )
            for n in range(2):
                ns = bass.ts(n, 512)
                for m in range(8):
                    ms = bass.ts(m, 128)
                    ps = pp_mm.tile([128, 512], F32, tag="mm", bufs=6)
                    for j in range(3):
                        nc.tensor.matmul(
                            ps,
                            wq8_sb[j][:, :, ms],
                            h8p[j][:, :, ns],
                            start=(j == 0),
                            stop=(j == 2),
                            perf_mode=DR,
                        )
                    dst = qTp[m // 2][:, m % 2, ns]
                    if (m * 2 + n) % 2 == 0:
                        nc.scalar.activation(
                            out=dst, in_=ps, func=AF.Identity,
                            scale=DRAIN_SCALE, bias=bq_sb[:, m : m + 1],
                        )
                    else:
                        nc.vector.tensor_scalar(
                            out=dst, in0=ps, scalar1=DRAIN_SCALE,
                            scalar2=bq_sb[:, m : m + 1], op0=ALU.mult, op1=ALU.add,
                        )

            def scores_block(b, pt):
                """S^T = kT.T @ qT for 512 queries; P^T = exp(S^T/512) fp8."""
                qs = bass.ts(b, 512)
                for kb in range(KB):
                    ps = pp_sc.tile([128, 512], F32, tag="sc", bufs=2)
                    for j in range(4):
                        nc.tensor.matmul(
                            ps,
                            kTp[j][:, :, kb * 128 : (kb + 1) * 128],
                            qTp[j][:, :, qs],
                            start=(j == 0),
                            stop=(j == 3),
                            perf_mode=DR,
                        )
                    nc.scalar.activation(
                        out=pt[:, kb, :], in_=ps, func=AF.Exp, scale=EXP_SCALE,
                    )

            pt_pool = tc.alloc_tile_pool(name="pt", bufs=1, side="right")
            pt = pt_pool.tile([128, KB, 512], FP8, name="pt")
            scores_block(0, pt)

            # V (token-major, fp8 DoubleRow on dithered wv8, x4 storage):
            # no bias (bv rides in wm's ones row)
            wv8_sb = load_w8(wv8, 0)  # reuses wk8's buffers once kT drains
            wmm_sb = []
            for i in range(5):
                i0, isz = i * 128, HID_CH_AUG[i]
                t = stream.tile([128, D], BF16, tag=f"wm_{i}", name=f"wm_{i}")
                nc.sync.dma_start(out=t[:isz], in_=wm[i0 : i0 + isz, :])
                wmm_sb.append(t)
            for m in range(16):
                ms = bass.ts(m, 128)
                for n in range(2):
                    ns = bass.ts(n, 512)
                    ps = pp_mm.tile([128, 512], F32, tag="mm", bufs=6)
                    for j in range(3):
                        nc.tensor.matmul(
                            ps,
                            h8p[j][:, :, ms],
                            wv8_sb[j][:, :, ns],
                            start=(j == 0),
                            stop=(j == 2),
                            perf_mode=DR,
                        )
                    dst = vp[m // 2][:, m % 2, ns]
                    if (m * 2 + n) % 2 == 0:
                        nc.vector.tensor_scalar_mul(dst, ps, DRAIN_SCALE)
                    else:
                        nc.scalar.mul(dst, ps, DRAIN_SCALE)
            # xmod (token-major, own half, bf16) + (bm+bv) via ones row
            for m in range(8):
                ms = bass.ts(m, 128)
                for n in range(2):
                    ns = bass.ts(n, 512)
                    ps = pp_mm.tile([128, 512], F32, tag="mm", bufs=6)
                    for kk in range(5):
                        ksz = HID_CH_AUG[kk]
                        nc.tensor.matmul(
                            ps,
                            hT_sb[kk][:ksz, ms],
                            wmm_sb[kk][:ksz, ns],
                            start=(kk == 0),
                            stop=(kk == 4),
                        )
                    nc.vector.tensor_copy(xm_sb[m][:, ns], ps)

            stream.release()
            hT_pool.release()
            pp_mm.release()

            # ---------- phase 2: attention ----------
            scratch = tc.alloc_tile_pool(name="scratch", bufs=1, side="right")
            tmpA = scratch.tile([128, D], BF16, name="tmpA")
            tmpB = scratch.tile([128, D], BF16, name="tmpB")
            tmpC = scratch.tile([128, D], BF16, name="tmpC")
            bf2_b = scratch.tile([128, D], BF16, name="bf2_b")
            nc.gpsimd.dma_start(out=bf2_b, in_=bcast_ap(bf2, D))
            g1_b = scratch.tile([128, D], BF16, name="g1_b")
            be1_b = scratch.tile([128, D], BF16, name="be1_b")
            g2_b = scratch.tile([128, D], BF16, name="g2_b")
            be2_b = scratch.tile([128, D], BF16, name="be2_b")
            if not identity_affine:
                nc.gpsimd.dma_start(out=g1_b, in_=bcast_ap(g1, D))
                nc.gpsimd.dma_start(out=be1_b, in_=bcast_ap(be1, D))
                nc.gpsimd.dma_start(out=g2_b, in_=bcast_ap(g2, D))
                nc.gpsimd.dma_start(out=be2_b, in_=bcast_ap(be2, D))
            # beA = be1 + bf2: the residual-side LN1 affine offset (the FFN
            # side has g1/be1 folded into wf18/bf1g on the host)
            beA_b = scratch.tile([128, D], BF16, name="beA_b")
            if not identity_affine:
                nc.vector.tensor_add(beA_b, be1_b, bf2_b)
            # x1pre scratch ring (4 live per attn block) and affined x1
            xp_t = [scratch.tile([128, D], BF16, tag="xp", name=f"xp_{i}", bufs=4) for i in range(8)]
            xa_sb = [scratch.tile([128, D], BF16, tag=f"xa_{i}", name=f"xa_{i}") for i in range(8)]

            wf_pool = tc.alloc_tile_pool(name="wf", bufs=1, side="right")
            wf18_sb = [wf_pool.tile([128, 2, D], FP8, tag=f"wf18_{j}", name=f"wf18_{j}") for j in range(4)]
            wf28_sb = [wf_pool.tile([128, 2, D], FP8, tag=f"wf28_{j}", name=f"wf28_{j}") for j in range(4)]
            for j in range(4):
                nc.sync.dma_start(out=wf18_sb[j], in_=wf18[j])

            pp_at = tc.alloc_tile_pool(name="psum_at", bufs=1, space="PSUM")
            pp_rs = tc.alloc_tile_pool(name="psum_rs", bufs=1, space="PSUM")

            def attn_block(b, pt):
                for qc in range(4):
                    qi = b * 4 + qc
                    ms = qc * 128
                    ps0 = pp_at.tile([128, 512], F32, tag="at", bufs=3)
                    ps1 = pp_at.tile([128, 512], F32, tag="at", bufs=3)
                    # per-qc rowsum tile: a whole PSUM bank, so the DVE read
                    # below never shares a bank with the next qc's PE writes.
                    rs = pp_rs.tile([128, 2], F32, tag="rs", bufs=2)
                    for jb in range(8):
                        lhsT = pt[:, 2 * jb : 2 * jb + 2, ms : ms + 128]
                        nc.tensor.matmul(
                            ps0, lhsT, vp[jb][:, :, 0:512],
                            start=(jb == 0), stop=(jb == 7), perf_mode=DR,
                        )
                        nc.tensor.matmul(
                            ps1, lhsT, vp[jb][:, :, 512:1024],
                            start=(jb == 0), stop=(jb == 7), perf_mode=DR,
                        )
                        nc.tensor.matmul(
                            rs, lhsT, ones8,
                            start=(jb == 0), stop=(jb == 7), perf_mode=DR,
                        )
                    nc.vector.reciprocal(rercp4[:, qc : qc + 1], rs[:, 0:1])
                    xp = xp_t[qi]
                    # x1pre = attn/rowsum + xmod; accum gives the LN mean sum
                    nc.vector.scalar_tensor_tensor(
                        out=xp[:, 0:512], in0=ps0,
                        scalar=rercp4[:, qc : qc + 1], in1=xm_sb[qi][:, 0:512],
                        op0=ALU.mult, op1=ALU.add, accum_out=sa0,
                    )
                    nc.vector.scalar_tensor_tensor(
                        out=xp[:, 512:1024], in0=ps1,
                        scalar=rercp4[:, qc : qc + 1], in1=xm_sb[qi][:, 512:1024],
                        op0=ALU.mult, op1=ALU.add, accum_out=sa1,
                    )
                    nc.vector.tensor_add(musum, sa0, sa1)
                    nc.vector.tensor_scalar_mul(mu4[:, qc : qc + 1], musum, 1.0 / D)
                    # sum of squares on scalar (vector is tighter here)
                    nc.scalar.activation(
                        out=tmpB, in_=xp, func=AF.Square,
                        accum_out=sq4[:, qc : qc + 1],
                    )
                    nc.vector.scalar_tensor_tensor(
                        out=msqe4[:, qc : qc + 1], in0=mu4[:, qc : qc + 1],
                        scalar=mu4[:, qc : qc + 1], in1=eps_t,
                        op0=ALU.mult, op1=ALU.subtract,
                    )
                    nc.vector.scalar_tensor_tensor(
                        out=ve4[:, qc : qc + 1], in0=sq4[:, qc : qc + 1],
                        scalar=1.0 / D, in1=msqe4[:, qc : qc + 1],
                        op0=ALU.mult, op1=ALU.subtract,
                    )
                # batched rstd for the 4 chunks (one act-table swap per block)
                nc.scalar.activation(out=std4, in_=ve4, func=AF.Sqrt)
                nc.vector.reciprocal(rstd4, std4)
                for qc in range(4):
                    qi = b * 4 + qc
                    # x1_sb holds t = (x1pre-mu)*rstd (pre-affine; feeds FFN)
                    nc.vector.tensor_scalar(
                        out=x1_sb[qi], in0=xp_t[qi],
                        scalar1=mu4[:, qc : qc + 1], scalar2=rstd4[:, qc : qc + 1],
                        op0=ALU.subtract, op1=ALU.mult,
                    )
                    # residual-side affine: xa = g1*t + (be1+bf2)
                    if identity_affine:
                        nc.gpsimd.tensor_add(xa_sb[qi], x1_sb[qi], bf2_b)
                    else:
                        nc.vector.tensor_mul(tmpC, x1_sb[qi], g1_b)
                        nc.vector.tensor_add(xa_sb[qi], tmpC, beA_b)

            def x1T_block(b, psum_pool, tp_bufs):
                for qc in range(4):
                    qi = b * 4 + qc
                    qoff = qi * 128
                    for g in range(2):
                        tp = psum_pool.tile([128, 512], BF16, tag="tp", bufs=tp_bufs)
                        for j in range(4):
                            dj = g * 4 + j
                            nc.tensor.transpose(
                                tp[:, j * 128 : (j + 1) * 128],
                                x1_sb[qi][:, dj * 128 : (dj + 1) * 128],
                                ident,
                            )
                        # drain with x4 prescale into fp8 for the FFN matmuls
                        nc.vector.tensor_scalar_mul(
                            x1T[:, g * 4 : (g + 1) * 4, qoff : qoff + 128],
                            tp.rearrange("p (g q) -> p g q", q=128),
                            QS,
                        )

            attn_block(0, pt)
            scores_block(1, pt)
            x1T_block(0, pp_at, 1)
            attn_block(1, pt)
            pp_rs.release()
            pp_at.release()
            pp_sc.release()
            kqvm.release()

            # ---------- phase 3: FFN (fp8 DoubleRow) + LN2 + relu ----------
            pp_f = tc.alloc_tile_pool(name="psum_f", bufs=1, space="PSUM")
            f1T_pool = tc.alloc_tile_pool(name="f1T", bufs=2, side="left")
            ffn_t = tc.alloc_tile_pool(name="ffn_t", bufs=2, side="left")
            for j in range(4):
                nc.sync.dma_start(out=wf28_sb[j], in_=wf28[j])

            def ffn1(nch):
                f1T_sb = f1T_pool.tile([128, 8, 512], FP8, tag="f1T")
                for m in range(8):
                    ms = bass.ts(m, 128)
                    ps = pp_f.tile([128, 512], F32, tag="f1", bufs=2)
                    for j in range(4):
                        nc.tensor.matmul(
                            ps,
                            wf18_sb[j][:, :, ms],
                            x1T[:, 2 * j : 2 * j + 2, nch * 512 : (nch + 1) * 512],
                            start=(j == 0),
                            stop=(j == 3),
                            perf_mode=DR,
                        )
                    # psum = 128*(t@wf1g); f1T = 4*relu(t@wf1g + bf1g)
                    nc.scalar.activation(
                        out=f1T_sb[:, m, :], in_=ps, func=AF.Relu,
                        scale=1.0 / 32.0, bias=bf1_sb[:, m : m + 1],
                    )
                return f1T_sb

            def ffn2_tq(qi, f1T_sb):
                    tq = qi % 4
                    x2a = ffn_t.tile([128, D], BF16, tag="x2a", bufs=3)
                    x2pre = ffn_t.tile([128, D], BF16, tag="x2pre", bufs=3)
                    ps = pp_f.tile([128, D], F32, tag="f2", bufs=2)
                    for dc in range(2):
                        ds_ = bass.ts(dc, 512)
                        for j in range(4):
                            nc.tensor.matmul(
                                ps[:, ds_],
                                f1T_sb[:, 2 * j : 2 * j + 2, tq * 128 : (tq + 1) * 128],
                                wf28_sb[j][:, :, ds_],
                                start=(j == 0),
                                stop=(j == 3),
                                perf_mode=DR,
                            )
                    # psum = 256*ffn; x2pre = ffn + (g1 t + be1 + bf2)
                    nc.vector.scalar_tensor_tensor(
                        out=x2pre, in0=ps, scalar=1.0 / 256.0,
                        in1=xa_sb[qi], op0=ALU.mult, op1=ALU.add,
                        accum_out=sa0,
                    )
                    nc.vector.tensor_scalar_mul(mu2, sa0, 1.0 / D)
                    # sum of squares on scalar: the vector STT drain is the
                    # tail's pacing op, keep vector light here
                    nc.scalar.activation(
                        out=x2a, in_=x2pre, func=AF.Square, accum_out=sqa,
                    )
                    nc.vector.scalar_tensor_tensor(
                        out=msqe2, in0=mu2, scalar=mu2, in1=eps_t,
                        op0=ALU.mult, op1=ALU.subtract,
                    )
                    nc.vector.scalar_tensor_tensor(
                        out=ve2, in0=sqa, scalar=1.0 / D, in1=msqe2,
                        op0=ALU.mult, op1=ALU.subtract,
                    )
                    nc.scalar.activation(out=std2, in_=ve2, func=AF.Sqrt)
                    nc.vector.reciprocal(rstd2, std2)
                    out_t = ffn_t.tile([128, D], F32, tag="out", bufs=3)
                    if identity_affine and tq % 2 == 1:
                        # vector path: normalize then relu-cast, balances the
                        # scalar engine which owns Square + the fused relu
                        tv = ffn_t.tile([128, D], BF16, tag="tv", bufs=2)
                        nc.vector.tensor_scalar(
                            out=tv, in0=x2pre, scalar1=mu2, scalar2=rstd2,
                            op0=ALU.subtract, op1=ALU.mult,
                        )
                        nc.vector.tensor_scalar_max(out_t, tv, 0.0)
                    elif identity_affine:
                        # y = relu((x2pre-mu2)*rstd2) fused into one activation
                        nmr2 = ffn_t.tile([128, 1], F32, tag="nmr", bufs=3)
                        nc.vector.tensor_scalar(
                            out=nmr2, in0=mu2, scalar1=rstd2, scalar2=-1.0,
                            op0=ALU.mult, op1=ALU.mult,
                        )
                        nc.scalar.activation(
                            out=out_t, in_=x2pre, func=AF.Relu,
                            scale=rstd2, bias=nmr2,
                        )
                    else:
                        t1 = ffn_t.tile([128, D], BF16, tag="t1", bufs=3)
                        t2 = ffn_t.tile([128, D], BF16, tag="t2", bufs=3)
                        nc.vector.tensor_scalar(
                            out=t1, in0=x2pre, scalar1=mu2, scalar2=rstd2,
                            op0=ALU.subtract, op1=ALU.mult,
                        )
                        nc.vector.tensor_mul(t2, t1, g2_b)
                        nc.vector.tensor_add(t1, t2, be2_b)
                        nc.scalar.activation(out=out_t, in_=t1, func=AF.Relu)
                    if qi % 2 == 0:
                        nc.scalar.dma_start(out=y[bass.ts(qi, 128), :], in_=out_t)
                    else:
                        nc.sync.dma_start(out=y[bass.ts(qi, 128), :], in_=out_t)

            # FFN1(1)'s PE stretch sits between the first tq epilogues so the
            # vector drains catch up without stalling later FFN2 matmuls
            f1T_0 = ffn1(0)
            x1T_block(1, pp_f, 2)
            ffn2_tq(0, f1T_0)
            ffn2_tq(1, f1T_0)
            f1T_1 = ffn1(1)
            ffn2_tq(2, f1T_0)
            ffn2_tq(3, f1T_0)
            for qi in range(4, 8):
                ffn2_tq(qi, f1T_1)

            pp_f.release()
            ffn_t.release()
            f1T_pool.release()
            wf_pool.release()
            scratch.release()
            pt_pool.release()
            x1T_pool.release()

    nc.finalize()
    return nc


_program_cache = {}


def _get_program(identity_affine):
    key = ("ia" if identity_affine else "gen",)
    if key not in _program_cache:
        _program_cache[key] = build_program(identity_affine)
    return _program_cache[key]


def _pack_w8(w, scale, rows):
    """[rows<=pairs*256, D] f32 -> [pairs, 128, 2, D] fp8 at `scale`."""
    pairs = (rows + 255) // 256
    arr = np.zeros((pairs * 256, D), np.float32)
    arr[: w.shape[0]] = w * scale
    arr = arr.reshape(pairs, 2, 128, D).transpose(0, 2, 1, 3)
    return np.ascontiguousarray(arr).astype(ml_dtypes.float8_e4m3)


_E4M3_ALL = np.arange(256, dtype=np.uint8).view(ml_dtypes.float8_e4m3).astype(np.float32)
_E4M3_GRID = np.sort(_E4M3_ALL[np.isfinite(_E4M3_ALL)])


def _dither_quant(w, scale):
    """Quantize w*scale to e4m3 f32 values, greedily flipping roundings so
    each column's quantization errors sum to ~0 (kills the coherent bias
    that attention averaging can't remove)."""
    ws = np.asarray(w, np.float32) * scale
    q = ws.astype(ml_dtypes.float8_e4m3).astype(np.float32)
    hi = np.searchsorted(_E4M3_GRID, q, side="left")
    out = q.copy()
    for d in range(ws.shape[1]):
        col = ws[:, d]
        qc = out[:, d]
        err = qc - col
        tot = err.sum()
        if tot > 0:
            alt = _E4M3_GRID[np.maximum(hi[:, d] - 1, 0)]
        else:
            alt = _E4M3_GRID[np.minimum(hi[:, d] + 1, len(_E4M3_GRID) - 1)]
        delta = alt - qc
        cost = np.abs(alt - col) - np.abs(err)
        for i in np.argsort(cost):
            if abs(tot) < 1e-4:
                break
            nt = tot + delta[i]
            if abs(nt) < abs(tot):
                qc[i] = alt[i]
                tot = nt
    return out / scale


def kernel(**inputs):
    from concourse.bass_utils import run_bass_kernel_spmd

    x = np.asarray(inputs["x"])  # [4, 2048, 768] f32
    bf = ml_dtypes.bfloat16

    g1 = np.asarray(inputs["g1"], np.float32)
    be1 = np.asarray(inputs["be1"], np.float32)
    g2 = np.asarray(inputs["g2"], np.float32)
    be2 = np.asarray(inputs["be2"], np.float32)
    # specialize when both LN affines are exactly identity (checked, not
    # assumed: the general program builds and runs otherwise)
    identity_affine = bool(
        np.all(g1 == 1.0) and np.all(be1 == 0.0)
        and np.all(g2 == 1.0) and np.all(be2 == 0.0)
    )
    wf1 = np.asarray(inputs["wf1"], np.float32)
    # fold the LN1 affine into the FFN first layer: x1@wf1 = t@(g1*wf1) + be1@wf1
    wf1g = g1[:, None] * wf1
    bf1g = (np.asarray(inputs["bf1"], np.float32) + be1 @ wf1) * QS

    shared = {
        "w_mlp": inputs["w_mlp"].astype(bf),
        "wq8": _pack_w8(np.asarray(inputs["wq"]), WS, HID),
        "wk8": _pack_w8(np.asarray(inputs["wk"]), WS, HID),
        "wv8": _pack_w8(_dither_quant(inputs["wv"], WS), WS, HID),
        # wm gets bm+bv so the on-chip hT ones-row adds the attention-path
        # bias for free
        "wm": np.vstack([inputs["wm"], (inputs["bm"] + inputs["bv"])[None, :]]).astype(bf),
        "wf18": _pack_w8(wf1g, 32.0, D),
        "wf28": _pack_w8(np.asarray(inputs["wf2"]), 64.0, D),
        "b_mlp": inputs["b_mlp"].astype(np.float32),
        "bq_s": (np.asarray(inputs["bq"]) * QS).astype(np.float32),
        "bk_s": (np.asarray(inputs["bk"]) * QS).astype(np.float32),
        "bf1g": bf1g.astype(np.float32),
        "bf2": inputs["bf2"].astype(bf),
        "g1": inputs["g1"].astype(bf),
        "be1": inputs["be1"].astype(bf),
        "g2": inputs["g2"].astype(bf),
        "be2": inputs["be2"].astype(bf),
    }

    in_maps = []
    for c in range(NCORES):
        b, half = c // 2, c % 2
        xb = np.roll(x[b], -Q * half, axis=0)  # own half first
        xT = np.ascontiguousarray(xb.T).astype(bf)  # [768, 2048]
        m = dict(shared)
        m["xT"] = xT
        in_maps.append(m)

    nc = _get_program(identity_affine)
    res = run_bass_kernel_spmd(nc, in_maps, core_ids=list(range(NCORES)))

    out = np.empty((B, S, D), np.float32)
    for c in range(NCORES):
        b, half = c // 2, c % 2
        out[b, half * Q : (half + 1) * Q, :] = res.results[c]["y"]
    return out
